# revision 7
# baseline (speedup 1.0000x reference)
"""ALSTM cell (attention-augmented LSTM) on 8 TRN2 NeuronCores.

Strategy: data-parallel over batch (B=256 -> 32 per core), weights
replicated, sequential scan local per shard (no collectives).

v2: each core runs TWO independent 16-batch recurrence streams (A/B),
software-pipelined half a step apart, so one stream's PE matmul blocks
fill the other stream's serial softmax/pointwise dependency chain.
The PE cost of the weight-streaming matmuls is batch-size independent
(moving operand = weight columns), so 2x16 costs the same PE time as
1x32 but hides nearly all chain latency.

Per-core layout (as in v1): recurrent state kept TRANSPOSED
(hT/cT: [u_within_chunk(128part), kchunk, b]); gate matmuls col-tiled
(gate j stationary at PE columns 32j -> PSUM partitions 32j..32j+16),
which lets disjoint column-tile matmuls dual-stream on the PE
(~2 rows/cycle observed). Softmax normalization rides the attention
transpose matmul as a diag(1/sum) moving operand. Sigmoid via tanh
half-angle with doubled state (U matrices pre-halved on host, output
un-doubled on host). Biases are all zero in this problem spec.

Phase order per macro-step t:
  front(A,t) | back2(B,t-1) | soft(A,t) | back1(A,t)
  | front(B,t) | back2(A,t) | soft(B,t) | back1(B,t)
front = x@Wa + h@Ua + h-part gate matmuls (PE, woven for col-tile
pairing); soft = tanh/exp/recip/diag; back1 = eT(diag) transpose +
ctxT + ctx-part gate matmuls; back2 = gate activation, transposes,
LSTM pointwise, out DMA.
"""

import sys

if "/opt/trn_rl_repo" not in sys.path:
    sys.path.append("/opt/trn_rl_repo")

from contextlib import ExitStack

import numpy as np

import concourse.bass as bass
import concourse.mybir as mybir
import concourse.tile as tile
from concourse.bass_utils import run_bass_kernel_spmd

F32 = mybir.dt.float32
BF16 = mybir.dt.bfloat16
AF = mybir.ActivationFunctionType

B, T, D, U = 256, 512, 256, 512
NCORES = 8
BS = B // NCORES  # 32 per core
BS2 = BS // 2  # 16 per stream
KU = U // 128  # 4 contraction chunks over h
KD = D // 128  # 2 contraction chunks over ctx/x
NG = 4  # gates i,f,o,c


def _split_excess_waits(nc: bass.Bass, max_waits: int = 1) -> None:
    """Move excess semaphore waits onto standalone EventSemaphore
    instructions (the BIR form of wait_ge). walrus' per-instruction
    descriptor has room for only ~one sync wait; Tile sometimes attaches
    more (slot-reuse WAR/WAW across engines). Splitting is sound: the
    engine executes the preceding waits in stream order."""
    k = 0
    for fn in nc.m.functions:
        for blk in fn.blocks:
            out = []
            for inst in blk.instructions:
                si = inst.sync_info
                if si is not None and len(si.on_wait) > max_waits:
                    waits = list(si.on_wait)
                    for w in waits[:-max_waits]:
                        k += 1
                        out.append(
                            mybir.InstEventSemaphore(
                                name=f"xwait-{k}",
                                engine=inst.engine,
                                ins=[],
                                outs=[],
                                sync_info=mybir.SyncInfo(
                                    on_wait=[w], on_update=[]
                                ),
                            )
                        )
                    inst.sync_info = mybir.SyncInfo(
                        on_wait=waits[-max_waits:],
                        on_update=list(si.on_update),
                    )
                out.append(inst)
            blk.instructions = out


class Stream:
    """Per-stream (A/B) tiles and emission helpers."""

    def __init__(self, nc, tc, ctx, name, bofs, weights, out_dram):
        self.nc = nc
        self.name = name
        self.bofs = bofs  # 0 or BS2 within the shared x tile
        self.w = weights
        self.out_dram = out_dram

        st = ctx.enter_context(tc.tile_pool(name=f"st{name}", bufs=1))
        self.hT = st.tile([128, KU, BS2], BF16)
        nc.vector.memset(self.hT[:], 0.0)
        self.cT = st.tile([128, KU, BS2], F32)
        nc.vector.memset(self.cT[:], 0.0)

        self.ps_att = ctx.enter_context(
            tc.tile_pool(name=f"psa{name}", bufs=1, space="PSUM")
        )
        self.ps_g = ctx.enter_context(
            tc.tile_pool(name=f"psg{name}", bufs=1, space="PSUM")
        )
        self.ps_awt = ctx.enter_context(
            tc.tile_pool(name=f"psw{name}", bufs=1, space="PSUM")
        )
        self.ps_gt = ctx.enter_context(
            tc.tile_pool(name=f"pst{name}", bufs=1, space="PSUM")
        )
        self.smp = ctx.enter_context(tc.tile_pool(name=f"smp{name}", bufs=2))
        self.gp = ctx.enter_context(tc.tile_pool(name=f"gp{name}", bufs=2))
        self.hp = ctx.enter_context(tc.tile_pool(name=f"hp{name}", bufs=4))
        self.cp = ctx.enter_context(tc.tile_pool(name=f"cp{name}", bufs=2))

        # Zero the full gates psum bank once: gact reads all 128
        # partitions, only 32j..32j+16 are ever written by matmuls.
        g0 = self.ps_g.tile([128, U], F32, name=f"g{name}")
        nc.vector.memset(g0[:], 0.0)
        self.gates_ps = g0

    # ---- phases ----

    def front(self, t, xt):
        """x@Wa + h@Ua (att psum) and h-part gate matmuls, woven so
        adjacent matmuls sit on different PE column tiles (att/xWa on
        cols 0-31 pair against gates g1..g3; g0 shares cols 0-31)."""
        nc = self.nc
        w = self.w
        att = self.ps_att.tile([BS2, D], F32, name=f"att{self.name}")
        self.att_ps = att
        gates = self.gates_ps
        xsl = slice(self.bofs, self.bofs + BS2)

        def xwa(kc, start):
            nc.tensor.matmul(
                att[:],
                xt[:, kc, xsl],
                w["Wa"][:, kc, :],
                start=start,
                stop=False,
            )

        def atth(kc, stop):
            nc.tensor.matmul(
                att[:],
                self.hT[:, kc, :],
                w["Ua"][:, kc, :],
                start=False,
                stop=stop,
            )

        def gh(g, kc):
            nc.tensor.matmul(
                gates[32 * g : 32 * g + BS2, :],
                self.hT[:, kc, :],
                w["Uall"][:, kc, 512 * g : 512 * (g + 1)],
                start=(kc == 0),
                stop=False,
                tile_position=(0, 32 * g),
            )

        xwa(0, True)
        gh(1, 0)
        xwa(1, False)
        gh(2, 0)
        atth(0, False)
        gh(3, 0)
        atth(1, False)
        gh(1, 1)
        atth(2, False)
        gh(2, 1)
        atth(3, True)
        gh(3, 1)
        gh(0, 0)
        gh(1, 2)
        gh(0, 1)
        gh(2, 2)
        gh(0, 2)
        gh(3, 2)
        gh(0, 3)
        gh(1, 3)
        gh(2, 3)
        gh(3, 3)

    def soft(self, t):
        nc = self.nc
        att_t = self.smp.tile([BS2, D], F32, name=f"at{self.name}")
        nc.scalar.activation(att_t[:], self.att_ps[:], AF.Tanh)
        att_e = self.smp.tile([BS2, D], BF16, name=f"ae{self.name}")
        esum = self.smp.tile([BS2, 1], F32, name=f"es{self.name}")
        nc.scalar.activation(att_e[:], att_t[:], AF.Exp, accum_out=esum[:])
        rsum = self.smp.tile([BS2, 1], F32, name=f"rs{self.name}")
        nc.vector.reciprocal(rsum[:], esum[:])
        diag = self.smp.tile([BS2, BS2], BF16, name=f"dg{self.name}")
        nc.gpsimd.tensor_scalar_mul(diag[:], self.w["I16"][:], rsum[:])
        self.att_e = att_e
        self.diag = diag

    def back1(self, t, xtv):
        """eT-with-diag transpose matmuls, ctxT, ctx-part gate matmuls."""
        nc = self.nc
        gates = self.gates_ps
        awt = self.ps_awt.tile([128, KD, BS2], F32, name=f"aw{self.name}")
        for kc in range(KD):
            nc.tensor.matmul(
                awt[:, kc, :],
                self.att_e[:, 128 * kc : 128 * (kc + 1)],
                self.diag[:],
                start=True,
                stop=True,
            )
        ctxT = self.smp.tile([128, KD, BS2], BF16, name=f"cx{self.name}")
        xsl = slice(self.bofs, self.bofs + BS2)
        nc.vector.tensor_mul(ctxT[:], awt[:], xtv[:, :, xsl])

        # ctx-part gate matmuls, split into u-column halves so the gate
        # activation (and then the transposes) can start on half 0 while
        # half 1 is still streaming on the PE.
        for uh in range(2):
            for kc in range(KD):
                for g in (1, 2, 3, 0):
                    nc.tensor.matmul(
                        gates[
                            32 * g : 32 * g + BS2,
                            256 * uh : 256 * (uh + 1),
                        ],
                        ctxT[:, kc, :],
                        self.w["Wall"][
                            :, kc, 512 * g + 256 * uh : 512 * g + 256 * (uh + 1)
                        ],
                        start=False,
                        stop=(kc == KD - 1),
                        tile_position=(0, 32 * g),
                        skip_group_check=True,
                    )

    def back2(self, t):
        """Gate activation, transposes to [u,b], LSTM pointwise, out."""
        nc = self.nc
        w = self.w
        gact = self.gp.tile([128, U], BF16, name=f"ga{self.name}")
        gt = self.ps_gt.tile([128, KU, 128], BF16, name=f"gt{self.name}")
        for uh in range(2):
            hsl = slice(256 * uh, 256 * (uh + 1))
            nc.scalar.activation(
                gact[:, hsl], self.gates_ps[:, hsl], AF.Tanh, scale=w["sc"][:]
            )
            for uc in (2 * uh, 2 * uh + 1):
                nc.tensor.transpose(
                    gt[:, uc, :], gact[:, 128 * uc : 128 * (uc + 1)], w["I128"][:]
                )
        iT = gt[:, :, 0:BS2]
        fT = gt[:, :, 32 : 32 + BS2]
        oT = gt[:, :, 64 : 64 + BS2]
        chT = gt[:, :, 96 : 96 + BS2]

        ch_sb = self.smp.tile([128, KU, BS2], BF16, name=f"ch{self.name}")
        nc.vector.tensor_copy(ch_sb[:], chT)
        t2 = self.smp.tile([128, KU, BS2], F32, name=f"t2{self.name}")
        nc.vector.scalar_tensor_tensor(
            t2[:], fT, 1.0, self.cT[:], mybir.AluOpType.add, mybir.AluOpType.mult
        )
        t1 = self.smp.tile([128, KU, BS2], F32, name=f"t1{self.name}")
        nc.vector.scalar_tensor_tensor(
            t1[:], iT, 1.0, ch_sb[:], mybir.AluOpType.add, mybir.AluOpType.mult
        )
        cT_new = self.cp.tile([128, KU, BS2], F32, name=f"c{self.name}")
        nc.vector.scalar_tensor_tensor(
            cT_new[:], t2[:], 0.5, t1[:], mybir.AluOpType.mult, mybir.AluOpType.add
        )
        ctanh = self.smp.tile([128, KU, BS2], BF16, name=f"ct{self.name}")
        nc.scalar.activation(ctanh[:], cT_new[:], AF.Tanh, scale=0.5)
        hT_new = self.hp.tile([128, KU, BS2], BF16, name=f"h{self.name}")
        nc.vector.scalar_tensor_tensor(
            hT_new[:], oT, 1.0, ctanh[:], mybir.AluOpType.add, mybir.AluOpType.mult
        )
        nc.sync.dma_start(self.out_dram[t], hT_new[:])
        self.hT = hT_new
        self.cT = cT_new


def build_nc(t_steps: int = T) -> bass.Bass:
    nc = bass.Bass()
    xTt = nc.declare_dram_parameter("xTt", [t_steps, 128, KD, BS], BF16, isOutput=False)
    Uall = nc.declare_dram_parameter("Uall", [U, NG * U], BF16, isOutput=False)
    Wall = nc.declare_dram_parameter("Wall", [D, NG * U], BF16, isOutput=False)
    Ua = nc.declare_dram_parameter("Ua", [U, D], BF16, isOutput=False)
    Wa = nc.declare_dram_parameter("Wa", [D, D], BF16, isOutput=False)
    I16 = nc.declare_dram_parameter("I16", [BS2, BS2], BF16, isOutput=False)
    I128 = nc.declare_dram_parameter("I128", [128, 128], BF16, isOutput=False)
    outA = nc.declare_dram_parameter(
        "outA", [t_steps, 128, KU, BS2], BF16, isOutput=True
    )
    outB = nc.declare_dram_parameter(
        "outB", [t_steps, 128, KU, BS2], BF16, isOutput=True
    )

    with ExitStack() as ctx:
        tc = ctx.enter_context(tile.TileContext(nc))
        wp = ctx.enter_context(tc.tile_pool(name="wp", bufs=1))
        Uall_sb = wp.tile([128, KU, NG * U], BF16)
        for kc in range(KU):
            nc.sync.dma_start(Uall_sb[:, kc, :], Uall[128 * kc : 128 * (kc + 1), :])
        Wall_sb = wp.tile([128, KD, NG * U], BF16)
        for kc in range(KD):
            nc.sync.dma_start(Wall_sb[:, kc, :], Wall[128 * kc : 128 * (kc + 1), :])
        Ua_sb = wp.tile([128, KU, D], BF16)
        for kc in range(KU):
            nc.sync.dma_start(Ua_sb[:, kc, :], Ua[128 * kc : 128 * (kc + 1), :])
        Wa_sb = wp.tile([128, KD, D], BF16)
        for kc in range(KD):
            nc.sync.dma_start(Wa_sb[:, kc, :], Wa[128 * kc : 128 * (kc + 1), :])
        I16_sb = wp.tile([BS2, BS2], BF16)
        nc.sync.dma_start(I16_sb[:], I16[:])
        I128_sb = wp.tile([128, 128], BF16)
        nc.sync.dma_start(I128_sb[:], I128[:])

        st = ctx.enter_context(tc.tile_pool(name="st", bufs=1))
        sc = st.tile([128, 1], F32)
        nc.vector.memset(sc[0:96, :], 0.5)
        nc.vector.memset(sc[96:128, :], 1.0)

        weights = {
            "Uall": Uall_sb,
            "Wall": Wall_sb,
            "Ua": Ua_sb,
            "Wa": Wa_sb,
            "I16": I16_sb,
            "I128": I128_sb,
            "sc": sc,
        }

        xp = ctx.enter_context(tc.tile_pool(name="xp", bufs=4))

        A = Stream(nc, tc, ctx, "A", 0, weights, outA)
        Bs = Stream(nc, tc, ctx, "B", BS2, weights, outB)

        def xdma(t):
            # shared x_t tiles: one copy for the PE (xWa lhsT), one for
            # the DVE (ctxT multiply) to keep DMA WAR fan-in small.
            xt = xp.tile([128, KD, BS], BF16, name="xt")
            nc.sync.dma_start(xt[:], xTt[t])
            xtv = xp.tile([128, KD, BS], BF16, name="xtv")
            nc.sync.dma_start(xtv[:], xTt[t])
            return xt, xtv

        # Symmetric software pipeline with half-step period: each
        # stream's softmax/pointwise latency is covered by the other
        # stream's front block, which the in-order PE queue reaches
        # first. Per half: front(S') | back1(S) | back2(S) | soft(S').
        xts = {0: xdma(0)}
        A.front(0, xts[0][0])
        A.soft(0)
        for t in range(t_steps):
            Bs.front(t, xts[t][0])
            A.back1(t, xts[t][1])
            A.back2(t)
            Bs.soft(t)
            if t + 1 < t_steps:
                xts[t + 1] = xdma(t + 1)
                A.front(t + 1, xts[t + 1][0])
            Bs.back1(t, xts[t][1])
            Bs.back2(t)
            if t + 1 < t_steps:
                A.soft(t + 1)
            xts.pop(t)

    _split_excess_waits(nc)
    return nc


def make_in_maps(x, W_i, U_i, W_f, U_f, W_o, U_o, W_c, U_c, W_a, U_a, t_steps=T):
    import ml_dtypes

    bf = ml_dtypes.bfloat16
    Uall = np.ascontiguousarray(
        np.concatenate([U_i, U_f, U_o, U_c], axis=1) * 0.5, bf
    )
    Wall = np.ascontiguousarray(np.concatenate([W_i, W_f, W_o, W_c], axis=1), bf)
    I16 = np.eye(BS2, dtype=bf)
    I128 = np.eye(128, dtype=bf)
    in_maps = []
    for i in range(NCORES):
        xs = np.asarray(x[BS * i : BS * (i + 1), :t_steps])  # [32, T, 256]
        # xTt[t, p, kc, b] = x[b, t, kc*128 + p]
        xTt = np.ascontiguousarray(
            xs.transpose(1, 2, 0).reshape(t_steps, KD, 128, BS).transpose(0, 2, 1, 3),
            bf,
        )
        in_maps.append(
            {
                "xTt": xTt,
                "Uall": Uall,
                "Wall": Wall,
                "Ua": np.ascontiguousarray(U_a * 0.5, bf),
                "Wa": np.ascontiguousarray(W_a, bf),
                "I16": I16,
                "I128": I128,
            }
        )
    return in_maps


def run(inputs, t_steps=T, trace=False, **spmd_kwargs):
    nc = build_nc(t_steps)
    in_maps = make_in_maps(
        inputs["x"],
        inputs["W_i"], inputs["U_i"],
        inputs["W_f"], inputs["U_f"],
        inputs["W_o"], inputs["U_o"],
        inputs["W_c"], inputs["U_c"],
        inputs["W_a"], inputs["U_a"],
        t_steps=t_steps,
    )
    res = run_bass_kernel_spmd(
        nc, in_maps, core_ids=list(range(NCORES)), trace=trace, **spmd_kwargs
    )
    outs = []
    for r in res.results:
        # out[t, p, uc, b] holds 2*h; u = uc*128 + p
        blocks = []
        for key in ("outA", "outB"):
            o = np.asarray(r[key]).astype(np.float32) * 0.5
            o = o.transpose(3, 0, 2, 1).reshape(BS2, t_steps, U)
            blocks.append(o)
        outs.append(np.concatenate(blocks, axis=0))  # [32, T, U]
    full = np.concatenate(outs, axis=0)
    return full, res


def kernel(**inputs) -> np.ndarray:
    full, _ = run(inputs)
    return full.astype(np.float32)


# revision 14
# speedup vs baseline: 1.0894x; 1.0894x over previous
"""ALSTM cell (attention-augmented LSTM) on 8 TRN2 NeuronCores.

Strategy: data-parallel over batch (B=256 -> 32 per core), weights
replicated, sequential scan local per shard (no collectives).

v2: each core runs TWO independent 16-batch recurrence streams (A/B),
software-pipelined half a step apart, so one stream's PE matmul blocks
fill the other stream's serial softmax/pointwise dependency chain.
The PE cost of the weight-streaming matmuls is batch-size independent
(moving operand = weight columns), so 2x16 costs the same PE time as
1x32 but hides nearly all chain latency.

Per-core layout (as in v1): recurrent state kept TRANSPOSED
(hT/cT: [u_within_chunk(128part), kchunk, b]); gate matmuls col-tiled
(gate j stationary at PE columns 32j -> PSUM partitions 32j..32j+16),
which lets disjoint column-tile matmuls dual-stream on the PE
(~2 rows/cycle observed). Softmax normalization rides the attention
transpose matmul as a diag(1/sum) moving operand. Sigmoid via tanh
half-angle with doubled state (U matrices pre-halved on host, output
un-doubled on host). Biases are all zero in this problem spec.

Phase order per macro-step t:
  front(A,t) | back2(B,t-1) | soft(A,t) | back1(A,t)
  | front(B,t) | back2(A,t) | soft(B,t) | back1(B,t)
front = x@Wa + h@Ua + h-part gate matmuls (PE, woven for col-tile
pairing); soft = tanh/exp/recip/diag; back1 = eT(diag) transpose +
ctxT + ctx-part gate matmuls; back2 = gate activation, transposes,
LSTM pointwise, out DMA.
"""

import sys

if "/opt/trn_rl_repo" not in sys.path:
    sys.path.append("/opt/trn_rl_repo")

from contextlib import ExitStack

import numpy as np

import concourse.bass as bass
import concourse.mybir as mybir
import concourse.tile as tile
from concourse.bass_utils import run_bass_kernel_spmd

F32 = mybir.dt.float32
BF16 = mybir.dt.bfloat16
AF = mybir.ActivationFunctionType

B, T, D, U = 256, 512, 256, 512
NCORES = 8
BS = B // NCORES  # 32 per core
BS2 = BS // 2  # 16 per stream
KU = U // 128  # 4 contraction chunks over h
KD = D // 128  # 2 contraction chunks over ctx/x
NG = 4  # gates i,f,o,c


def _split_excess_waits(nc: bass.Bass, max_waits: int = 1) -> None:
    """Move excess semaphore waits onto standalone EventSemaphore
    instructions (the BIR form of wait_ge). walrus' per-instruction
    descriptor has room for only ~one sync wait; Tile sometimes attaches
    more (slot-reuse WAR/WAW across engines). Splitting is sound: the
    engine executes the preceding waits in stream order."""
    k = 0
    for fn in nc.m.functions:
        for blk in fn.blocks:
            out = []
            for inst in blk.instructions:
                si = inst.sync_info
                if si is not None and len(si.on_wait) > max_waits:
                    waits = list(si.on_wait)
                    for w in waits[:-max_waits]:
                        k += 1
                        out.append(
                            mybir.InstEventSemaphore(
                                name=f"xwait-{k}",
                                engine=inst.engine,
                                ins=[],
                                outs=[],
                                sync_info=mybir.SyncInfo(
                                    on_wait=[w], on_update=[]
                                ),
                            )
                        )
                    inst.sync_info = mybir.SyncInfo(
                        on_wait=waits[-max_waits:],
                        on_update=list(si.on_update),
                    )
                out.append(inst)
            blk.instructions = out


class Stream:
    """Per-stream (A/B) tiles and emission helpers."""

    def __init__(self, nc, tc, ctx, name, bofs, weights, out_dram):
        self.nc = nc
        self.name = name
        self.bofs = bofs  # 0 or BS2 within the shared x tile
        self.w = weights
        self.out_dram = out_dram

        st = ctx.enter_context(tc.tile_pool(name=f"st{name}", bufs=1))
        self.hT = st.tile([128, KU, BS2], BF16)
        nc.vector.memset(self.hT[:], 0.0)
        self.cT = st.tile([128, KU, BS2], F32)
        nc.vector.memset(self.cT[:], 0.0)

        self.ps_att = ctx.enter_context(
            tc.tile_pool(name=f"psa{name}", bufs=1, space="PSUM")
        )
        self.ps_g = ctx.enter_context(
            tc.tile_pool(name=f"psg{name}", bufs=1, space="PSUM")
        )
        self.ps_awt = ctx.enter_context(
            tc.tile_pool(name=f"psw{name}", bufs=1, space="PSUM")
        )
        self.ps_gt = ctx.enter_context(
            tc.tile_pool(name=f"pst{name}", bufs=1, space="PSUM")
        )
        self.smp = ctx.enter_context(tc.tile_pool(name=f"smp{name}", bufs=2))
        self.gp = ctx.enter_context(tc.tile_pool(name=f"gp{name}", bufs=2))
        self.hp = ctx.enter_context(tc.tile_pool(name=f"hp{name}", bufs=4))
        self.cp = ctx.enter_context(tc.tile_pool(name=f"cp{name}", bufs=2))

        # Zero the full gates psum bank once: gact reads all 128
        # partitions, only 32j..32j+16 are ever written by matmuls.
        g0 = self.ps_g.tile([128, U], F32, name=f"g{name}")
        nc.vector.memset(g0[:], 0.0)
        self.gates_ps = g0

    # ---- phases ----

    def front(self, t, xt):
        """x@Wa + h@Ua (att psum) and h-part gate matmuls, woven so
        adjacent matmuls sit on different PE column tiles (att/xWa on
        cols 0-31 pair against gates g1..g3; g0 shares cols 0-31)."""
        nc = self.nc
        w = self.w
        att = self.ps_att.tile([BS2, D], F32, name=f"att{self.name}")
        self.att_ps = att
        gates = self.gates_ps
        xsl = slice(self.bofs, self.bofs + BS2)

        def xwa(kc, start):
            nc.tensor.matmul(
                att[:],
                xt[:, kc, xsl],
                w["Wa"][:, kc, :],
                start=start,
                stop=False,
            )

        def atth(kc, stop):
            nc.tensor.matmul(
                att[:],
                self.hT[:, kc, :],
                w["Ua"][:, kc, :],
                start=False,
                stop=stop,
            )

        def gh(g, kc):
            nc.tensor.matmul(
                gates[32 * g : 32 * g + BS2, :],
                self.hT[:, kc, :],
                w["Uall"][:, kc, 512 * g : 512 * (g + 1)],
                start=(kc == 0),
                stop=False,
                tile_position=(0, 32 * g),
            )

        # Weave: adjacent matmuls on different PE column tiles pair up
        # (~2 rows/cycle); h-chunk kc matmuls are gated on the uc=kc
        # half of the previous pointwise, so low chunks go first.
        xwa(0, True)
        gh(1, 0)
        xwa(1, False)
        gh(2, 0)
        atth(0, False)
        gh(3, 0)
        gh(0, 0)
        gh(1, 1)
        atth(1, False)
        gh(2, 1)
        gh(3, 1)
        gh(0, 1)
        atth(2, False)
        gh(1, 2)
        gh(2, 2)
        gh(3, 2)
        gh(0, 2)
        atth(3, True)
        gh(1, 3)
        gh(2, 3)
        gh(3, 3)
        gh(0, 3)

    def soft(self, t):
        nc = self.nc
        att_t = self.smp.tile([BS2, D], F32, name=f"at{self.name}")
        nc.scalar.activation(att_t[:], self.att_ps[:], AF.Tanh)
        att_e = self.smp.tile([BS2, D], BF16, name=f"ae{self.name}")
        esum = self.smp.tile([BS2, 1], F32, name=f"es{self.name}")
        nc.scalar.activation(att_e[:], att_t[:], AF.Exp, accum_out=esum[:])
        self.att_e = att_e
        self.esum = esum

    def back1(self, t, xtv):
        """Plain eT transpose (no normalization wait), 1/sum broadcast
        via rank-1 matmul, ctxT, ctx-part gate matmuls."""
        nc = self.nc
        w = self.w
        gates = self.gates_ps
        # One PSUM bank holds eT chunks, the 1/sum broadcast, and the
        # transposed accumulator. A single start=True (first matmul)
        # pending-zeroes the whole 2KB bank row; every later matmul's
        # first write to its own sub-region then overwrites, so they all
        # carry start=False (skip_group_check silences the group pairing
        # validation, which assumes one region per group).
        ab = self.ps_awt.tile([128, KD, 3, BS2], F32, name=f"ab{self.name}")
        awt = ab[:, :, 0, :]
        bcast = ab[:, :, 1, :]
        esumT = ab[0:1, :, 2, :]
        # unnormalized e^T chunks (start right after exp; the sum path
        # runs in parallel rather than in series).
        for kc in range(KD):
            nc.tensor.matmul(
                awt[:, kc, :],
                self.att_e[:, 128 * kc : 128 * (kc + 1)],
                w["I16"][:],
                start=(kc == 0),
                stop=False,
                skip_group_check=True,
            )
        # esumT[0, kc, b] = esum[b]: PE transpose of the accumulator,
        # duplicated per k-chunk so the broadcast matches ctxT's shape.
        for kc in range(KD):
            nc.tensor.matmul(
                esumT[:, kc, :],
                self.esum[:],
                w["I16f"][:],
                start=False,
                stop=False,
                skip_group_check=True,
            )
        rsumT = self.smp.tile([1, KD, BS2], F32, name=f"rt{self.name}")
        nc.vector.reciprocal(rsumT[:], esumT)
        nc.tensor.matmul(
            bcast, w["ones1"][:], rsumT[:],
            start=False, stop=True, skip_group_check=True,
        )

        tmp = self.smp.tile([128, KD, BS2], BF16, name=f"tm{self.name}")
        xsl = slice(self.bofs, self.bofs + BS2)
        nc.vector.tensor_mul(tmp[:], awt, xtv[:, :, xsl])
        ctxT = self.smp.tile([128, KD, BS2], BF16, name=f"cx{self.name}")
        nc.vector.tensor_mul(ctxT[:], tmp[:], bcast)

        for kc in range(KD):
            for g in (1, 2, 3, 0):
                nc.tensor.matmul(
                    gates[32 * g : 32 * g + BS2, :],
                    ctxT[:, kc, :],
                    self.w["Wall"][:, kc, 512 * g : 512 * (g + 1)],
                    start=False,
                    stop=(kc == KD - 1),
                    tile_position=(0, 32 * g),
                )

    def back2(self, t):
        """Gate activation, transposes to [u,b], LSTM pointwise, out."""
        nc = self.nc
        w = self.w
        gact = self.gp.tile([128, U], BF16, name=f"ga{self.name}")
        gt = self.ps_gt.tile([128, KU, 128], BF16, name=f"gt{self.name}")
        for uh in range(2):
            hsl = slice(256 * uh, 256 * (uh + 1))
            nc.scalar.activation(
                gact[:, hsl], self.gates_ps[:, hsl], AF.Tanh, scale=w["sc"][:]
            )
            for uc in (2 * uh, 2 * uh + 1):
                nc.tensor.transpose(
                    gt[:, uc, :], gact[:, 128 * uc : 128 * (uc + 1)], w["I128"][:]
                )
        # LSTM pointwise, split into uc halves: the next front's k-chunk
        # kc matmuls only need hT[:, kc, :], so publishing the low half
        # early lets the PE restart ~0.9us sooner.
        ch_sb = self.smp.tile([128, KU, BS2], BF16, name=f"ch{self.name}")
        t2 = self.smp.tile([128, KU, BS2], F32, name=f"t2{self.name}")
        t1 = self.smp.tile([128, KU, BS2], F32, name=f"t1{self.name}")
        cT_new = self.cp.tile([128, KU, BS2], F32, name=f"c{self.name}")
        ctanh = self.smp.tile([128, KU, BS2], BF16, name=f"ct{self.name}")
        hT_new = self.hp.tile([128, KU, BS2], BF16, name=f"h{self.name}")

        def u(x, h):
            return x[:, 2 * h : 2 * h + 2, :]

        def gv(ofs, h):
            return gt[:, 2 * h : 2 * h + 2, ofs : ofs + BS2]

        A = mybir.AluOpType.add
        M = mybir.AluOpType.mult
        V = nc.vector

        def pw_half(h):
            V.tensor_copy(u(ch_sb, h), gv(96, h))
            V.scalar_tensor_tensor(u(t2, h), gv(32, h), 1.0, u(self.cT, h), A, M)
            V.scalar_tensor_tensor(u(t1, h), gv(0, h), 1.0, u(ch_sb, h), A, M)
            V.scalar_tensor_tensor(u(cT_new, h), u(t2, h), 0.5, u(t1, h), M, A)

        pw_half(0)
        nc.scalar.activation(u(ctanh, 0), u(cT_new, 0), AF.Tanh, scale=0.5)
        pw_half(1)
        V.scalar_tensor_tensor(u(hT_new, 0), gv(64, 0), 1.0, u(ctanh, 0), A, M)
        nc.scalar.activation(u(ctanh, 1), u(cT_new, 1), AF.Tanh, scale=0.5)
        V.scalar_tensor_tensor(u(hT_new, 1), gv(64, 1), 1.0, u(ctanh, 1), A, M)
        nc.sync.dma_start(self.out_dram[t], hT_new[:])
        self.hT = hT_new
        self.cT = cT_new


def build_nc(t_steps: int = T) -> bass.Bass:
    nc = bass.Bass()
    xTt = nc.declare_dram_parameter("xTt", [t_steps, 128, KD, BS], BF16, isOutput=False)
    Uall = nc.declare_dram_parameter("Uall", [U, NG * U], BF16, isOutput=False)
    Wall = nc.declare_dram_parameter("Wall", [D, NG * U], BF16, isOutput=False)
    Ua = nc.declare_dram_parameter("Ua", [U, D], BF16, isOutput=False)
    Wa = nc.declare_dram_parameter("Wa", [D, D], BF16, isOutput=False)
    I16 = nc.declare_dram_parameter("I16", [BS2, BS2], BF16, isOutput=False)
    I128 = nc.declare_dram_parameter("I128", [128, 128], BF16, isOutput=False)
    outA = nc.declare_dram_parameter(
        "outA", [t_steps, 128, KU, BS2], BF16, isOutput=True
    )
    outB = nc.declare_dram_parameter(
        "outB", [t_steps, 128, KU, BS2], BF16, isOutput=True
    )

    with ExitStack() as ctx:
        tc = ctx.enter_context(tile.TileContext(nc))
        wp = ctx.enter_context(tc.tile_pool(name="wp", bufs=1))
        Uall_sb = wp.tile([128, KU, NG * U], BF16)
        for kc in range(KU):
            nc.sync.dma_start(Uall_sb[:, kc, :], Uall[128 * kc : 128 * (kc + 1), :])
        Wall_sb = wp.tile([128, KD, NG * U], BF16)
        for kc in range(KD):
            nc.sync.dma_start(Wall_sb[:, kc, :], Wall[128 * kc : 128 * (kc + 1), :])
        Ua_sb = wp.tile([128, KU, D], BF16)
        for kc in range(KU):
            nc.sync.dma_start(Ua_sb[:, kc, :], Ua[128 * kc : 128 * (kc + 1), :])
        Wa_sb = wp.tile([128, KD, D], BF16)
        for kc in range(KD):
            nc.sync.dma_start(Wa_sb[:, kc, :], Wa[128 * kc : 128 * (kc + 1), :])
        I16_sb = wp.tile([BS2, BS2], BF16)
        nc.sync.dma_start(I16_sb[:], I16[:])
        I128_sb = wp.tile([128, 128], BF16)
        nc.sync.dma_start(I128_sb[:], I128[:])
        I16f_sb = wp.tile([BS2, BS2], F32)
        nc.vector.tensor_copy(I16f_sb[:], I16_sb[:])

        st = ctx.enter_context(tc.tile_pool(name="st", bufs=1))
        sc = st.tile([128, 1], F32)
        nc.vector.memset(sc[0:96, :], 0.5)
        nc.vector.memset(sc[96:128, :], 1.0)
        ones1 = st.tile([1, 128], F32)
        nc.vector.memset(ones1[:], 1.0)

        weights = {
            "Uall": Uall_sb,
            "Wall": Wall_sb,
            "Ua": Ua_sb,
            "Wa": Wa_sb,
            "I16": I16_sb,
            "I16f": I16f_sb,
            "I128": I128_sb,
            "sc": sc,
            "ones1": ones1,
        }

        xp = ctx.enter_context(tc.tile_pool(name="xp", bufs=4))

        A = Stream(nc, tc, ctx, "A", 0, weights, outA)
        Bs = Stream(nc, tc, ctx, "B", BS2, weights, outB)

        def xdma(t):
            # shared x_t tiles: one copy for the PE (xWa lhsT), one for
            # the DVE (ctxT multiply) to keep DMA WAR fan-in small.
            xt = xp.tile([128, KD, BS], BF16, name="xt")
            nc.sync.dma_start(xt[:], xTt[t])
            xtv = xp.tile([128, KD, BS], BF16, name="xtv")
            nc.sync.dma_start(xtv[:], xTt[t])
            return xt, xtv

        # Symmetric software pipeline with half-step period: each
        # stream's softmax/pointwise latency is covered by the other
        # stream's front block, which the in-order PE queue reaches
        # first. Per half: front(S') | back1(S) | back2(S) | soft(S').
        xts = {0: xdma(0)}
        A.front(0, xts[0][0])
        A.soft(0)
        for t in range(t_steps):
            Bs.front(t, xts[t][0])
            A.back1(t, xts[t][1])
            A.back2(t)
            Bs.soft(t)
            if t + 1 < t_steps:
                xts[t + 1] = xdma(t + 1)
                A.front(t + 1, xts[t + 1][0])
            Bs.back1(t, xts[t][1])
            Bs.back2(t)
            if t + 1 < t_steps:
                A.soft(t + 1)
            xts.pop(t)

    _split_excess_waits(nc)
    return nc


def make_in_maps(x, W_i, U_i, W_f, U_f, W_o, U_o, W_c, U_c, W_a, U_a, t_steps=T):
    import ml_dtypes

    bf = ml_dtypes.bfloat16
    Uall = np.ascontiguousarray(
        np.concatenate([U_i, U_f, U_o, U_c], axis=1) * 0.5, bf
    )
    Wall = np.ascontiguousarray(np.concatenate([W_i, W_f, W_o, W_c], axis=1), bf)
    I16 = np.eye(BS2, dtype=bf)
    I128 = np.eye(128, dtype=bf)
    in_maps = []
    for i in range(NCORES):
        xs = np.asarray(x[BS * i : BS * (i + 1), :t_steps])  # [32, T, 256]
        # xTt[t, p, kc, b] = x[b, t, kc*128 + p]
        xTt = np.ascontiguousarray(
            xs.transpose(1, 2, 0).reshape(t_steps, KD, 128, BS).transpose(0, 2, 1, 3),
            bf,
        )
        in_maps.append(
            {
                "xTt": xTt,
                "Uall": Uall,
                "Wall": Wall,
                "Ua": np.ascontiguousarray(U_a * 0.5, bf),
                "Wa": np.ascontiguousarray(W_a, bf),
                "I16": I16,
                "I128": I128,
            }
        )
    return in_maps


def run(inputs, t_steps=T, trace=False, **spmd_kwargs):
    nc = build_nc(t_steps)
    in_maps = make_in_maps(
        inputs["x"],
        inputs["W_i"], inputs["U_i"],
        inputs["W_f"], inputs["U_f"],
        inputs["W_o"], inputs["U_o"],
        inputs["W_c"], inputs["U_c"],
        inputs["W_a"], inputs["U_a"],
        t_steps=t_steps,
    )
    res = run_bass_kernel_spmd(
        nc, in_maps, core_ids=list(range(NCORES)), trace=trace, **spmd_kwargs
    )
    outs = []
    for r in res.results:
        # out[t, p, uc, b] holds 2*h; u = uc*128 + p
        blocks = []
        for key in ("outA", "outB"):
            o = np.asarray(r[key]).astype(np.float32) * 0.5
            o = o.transpose(3, 0, 2, 1).reshape(BS2, t_steps, U)
            blocks.append(o)
        outs.append(np.concatenate(blocks, axis=0))  # [32, T, U]
    full = np.concatenate(outs, axis=0)
    return full, res


def kernel(**inputs) -> np.ndarray:
    full, _ = run(inputs)
    return full.astype(np.float32)
